# revision 35
# baseline (speedup 1.0000x reference)
"""Trainium2 Bass kernel for nn_Attention_14877766713476.

Causal multi-head attention with full-hidden RoPE:
  q,k,v = x@W{q,k,v} + b;  q,k = rope(q,k);  heads=16, hd=128;
  causal softmax attention;  out = attn@Wo + bo.

Sharding: tensor-parallel over heads across 8 cores. RoPE pairs hidden
column c with c +/- 1024, i.e. head h with head h+8 -- so core m owns
heads {m, m+8} and RoPE stays core-local. Each core computes its two
heads end-to-end and a partial output projection (rows of Wo); the host
sums the 8 partials.

All matmuls in bf16 with fp32 PSUM accumulation. Host pre-transposes
x -> xT (contraction dim on partitions) and pre-slices/casts weights,
so the device does zero transposes.

v3 changes over v2 (trace-driven):
  - prologue fans the first-need tiles (wq g0, xt c0 g0 halves, wkv g0)
    across all four DMA rings; first matmul ~11 us instead of ~20 us.
  - steady-state x chunks split sync+scalar, cos/sin on vector: no
    per-chunk arrival jitter.
  - attention phase: ScalarE runs ONLY exp (evictions 1-in-4 ACT /
    rest DVE, chunk-7 evictions DVE, presum tree on GpSimd), so exp
    never queues behind a copy and the trailing pv matmuls don't stall.
  - diagonal pairs use a single merged exp (stale-psum columns zeroed
    by the memsets after it); rowsum is one ones-matmul per group fed
    by a GpSimd binary-counter presum tree.
  - final fin drain keeps reserved group-14 projection units as PE
    filler; last group's output DMA ships per-512-col slice round-robin
    over all four rings.

Layouts (per core, host-prepared, all bf16 unless noted):
  xT    [128, 16*4096]  col = a*4096 + t   (d = a*128 + p, t = b*2048+s)
  wq/wk/wv [128, 16*256] col = a*256 + c   (d = a*128 + p, c in 0..255)
  wo    [128, 2*2048]   col = cb*2048 + dcol  (c = cb*128 + p)
  cosT/sinT [128, 2*4096] col = cb*4096 + t   (c = cb*128 + p; sinT block0
                           negated so rope_b = q_b*cos_b + q_{1-b}*sinT_b)
  tri   [128, 128]      tri[kj, qq] = (qq >= kj)  (intra-block causal)
  ones  [128, 128]      all ones (rowsum matmul stationary)
Output per core: out [4096, 2048] bf16 partial (this core's two heads
through Wo rows); host sums partials in fp32 and adds bv@Wo + bo.
"""

import math
from contextlib import ExitStack

import numpy as np
import ml_dtypes

N_CORES = 8
B, S, D, H = 2, 2048, 2048, 16
HD = D // H          # 128
T = B * S            # 4096
P = 128
NB = D // P          # 16 contraction blocks
NG = 4               # a-groups (DMA split granularity)
GA = NB // NG        # 4 a-blocks per group
TCH = 512            # token chunk (QKV phase free dim)
NCH = T // TCH       # 8
QBLK = 512           # query block (attention phase free dim)
NQ = S // QBLK       # 4 query blocks per (batch, head)
SCALE = 1.0 / math.sqrt(HD)

BF16 = ml_dtypes.bfloat16

_CACHE = {}
LAST_RESULTS = None


def _build_program():
    import concourse.tile as tile
    from concourse import bacc, mybir

    bf = mybir.dt.bfloat16
    f32 = mybir.dt.float32
    Act = mybir.ActivationFunctionType

    nc = bacc.Bacc("TRN2", target_bir_lowering=False, debug=False,
                   num_devices=N_CORES)

    # DRAM layouts are chunk-major so every DMA descriptor reads fully
    # contiguous per-partition rows (16 KB for x, 4 KB for cos/sin) --
    # fragmented rows cost one DMA packet per 1 KB segment and cap the
    # aggregate stream far below HBM bandwidth.
    xT = nc.dram_tensor("xT", [P, NCH * NB * TCH], bf,
                        kind="ExternalInput").ap()
    wq = nc.dram_tensor("wq", [P, NB * 256], bf, kind="ExternalInput").ap()
    wkv = nc.dram_tensor("wkv", [P, NG * 2 * GA * 256], bf,
                         kind="ExternalInput").ap()
    wo = nc.dram_tensor("wo", [P, 2 * D], bf, kind="ExternalInput").ap()
    cs = nc.dram_tensor("cs", [P, NCH * 4 * TCH], bf,
                        kind="ExternalInput").ap()
    tri = nc.dram_tensor("tri", [P, P], bf, kind="ExternalInput").ap()
    ones = nc.dram_tensor("ones", [P, P], bf, kind="ExternalInput").ap()
    out = nc.dram_tensor("out", [T, D], bf, kind="ExternalOutput").ap()

    xT5 = xT.rearrange("p (c g a t) -> p c g a t", c=NCH, g=NG, a=GA)
    xT4 = xT.rearrange("p (c b t) -> p c b t", c=NCH, b=NB)
    wq4 = wq.rearrange("p (g a c) -> p g a c", g=NG, a=GA)
    wkv5 = wkv.rearrange("p (g w a c) -> p g w a c", g=NG, w=2, a=GA)
    cs5 = cs.rearrange("p (c w k t) -> p c w k t", c=NCH, w=2, k=2)

    with tile.TileContext(nc) as tc, ExitStack() as ctx:
        const = ctx.enter_context(tc.tile_pool(name="const", bufs=1))
        persist = ctx.enter_context(tc.tile_pool(name="persist", bufs=1))
        xt_pool = ctx.enter_context(tc.tile_pool(name="xt", bufs=2))
        cs_pool = ctx.enter_context(tc.tile_pool(name="cs", bufs=2))
        raw_pool = ctx.enter_context(tc.tile_pool(name="raw", bufs=2))
        tmp_pool = ctx.enter_context(tc.tile_pool(name="tmp", bufs=4))
        exp_pool = ctx.enter_context(tc.tile_pool(name="exp", bufs=6))
        rec_pool = ctx.enter_context(tc.tile_pool(name="rec", bufs=2))
        orow_pool = ctx.enter_context(tc.tile_pool(name="orow", bufs=3))

        # Weight/x prologue: the first q accumulation chain needs only
        # wq g0 + xt c0 g0, so those split across four rings (sync,
        # scalar, vector, gpsimd) and everything else follows in
        # need-order. With ~120 GB/s per ring, 4-way fan-out gets the
        # first matmul issued ~2 us after the rings go live instead of
        # serializing 768 KB behind one queue.
        wq_sb = [const.tile([P, GA, 256], bf, tag=f"wq{g}", name=f"wq_sb{g}")
                 for g in range(NG)]
        wkv_sb = [const.tile([P, 2, GA, 256], bf, tag=f"wkv{g}",
                             name=f"wkv_sb{g}") for g in range(NG)]
        wk_sb = [wkv_sb[g][:, 0] for g in range(NG)]
        wv_sb = [wkv_sb[g][:, 1] for g in range(NG)]
        # the very first accumulation step needs only wq g0's a0 column
        # block and xt c0 a0-a1, so those lead their queues; the sync
        # queue goes live ~1.5 us before scalar, gpsimd last.
        xt_c0 = xt_pool.tile([P, NB, TCH], bf, tag="xt", name="xt_0")
        nc.sync.dma_start(wq_sb[0][:, 0:1], wq4[:, 0, 0:1])
        nc.scalar.dma_start(wq_sb[0][:, 1:GA], wq4[:, 0, 1:GA])
        nc.sync.dma_start(xt_c0[:, 0:2, :], xT5[:, 0, 0, 0:2])
        nc.gpsimd.dma_start(wkv_sb[0][:], wkv5[:, 0])
        nc.scalar.dma_start(xt_c0[:, 2:GA, :], xT5[:, 0, 0, 2:GA])
        nc.sync.dma_start(xt_c0[:, GA:2 * GA, :], xT5[:, 0, 1])
        nc.scalar.dma_start(wq_sb[1][:], wq4[:, 1])
        nc.gpsimd.dma_start(wkv_sb[1][:], wkv5[:, 1])
        nc.scalar.dma_start(xt_c0[:, 2 * GA:3 * GA, :], xT5[:, 0, 2])
        nc.sync.dma_start(wq_sb[2][:], wq4[:, 2])
        nc.gpsimd.dma_start(wkv_sb[2][:], wkv5[:, 2])
        nc.sync.dma_start(xt_c0[:, 3 * GA:4 * GA, :], xT5[:, 0, 3])
        nc.scalar.dma_start(wq_sb[3][:], wq4[:, 3])
        nc.gpsimd.dma_start(wkv_sb[3][:], wkv5[:, 3])

        cs0 = cs_pool.tile([P, 2, 2, TCH], bf, tag="cs", name="cs_0")
        nc.scalar.dma_start(cs0[:], cs5[:, 0])

        wo_sb = const.tile([P, 2 * D], bf, tag="wo")
        tri_sb = const.tile([P, P], bf, tag="tri")
        ones_sb = const.tile([P, P], bf, tag="ones")
        nc.gpsimd.dma_start(tri_sb[:], tri[:])
        nc.gpsimd.dma_start(ones_sb[:], ones[:])

        # persistent activations
        q_all = persist.tile([P, 2 * T], bf, tag="q_all")      # roped qT
        k_all = persist.tile([P, 2 * T], bf, tag="k_all")      # roped kT
        v_all = persist.tile([P, 32 * 256], bf, tag="v_all")   # v natural
        at_all = persist.tile([P, 2 * T], bf, tag="at_all")    # attnT

        # ---------------- Phase 1: QKV projections + RoPE ----------------
        # Only chunks 0-3 (batch 0) run as a dedicated phase; chunks 4-7
        # are emitted as PE-filler units inside the b=0 attention groups
        # below. That stretches the attention phase's ScalarE/DVE work
        # (exp, evictions, presums) over a ~220 us window instead of
        # ~140 us, so neither engine saturates and the PE never waits.
        with tc.tile_pool(name="psum1", bufs=4, space="PSUM") as psum:
            for tcix in range(4):
                t0 = tcix * TCH
                if tcix == 0:
                    xta = lambda a: xt_c0[:, a]
                    cosc, sinc = cs0[:, 0], cs0[:, 1]
                else:
                    # steady-state chunk stream split over both HWDGE
                    # rings (sync + scalar) with cos/sin on the gpsimd
                    # SWDGE ring: each ring carries ~half the 122 GB/s
                    # demand, so per-chunk arrival jitter never stalls
                    # the PE.
                    xtc = xt_pool.tile([P, NB, TCH], bf, tag="xt",
                                       name=f"xt_{tcix}")
                    nc.sync.dma_start(xtc[:, 0:NB // 2], xT4[:, tcix, 0:NB // 2])
                    nc.scalar.dma_start(xtc[:, NB // 2:], xT4[:, tcix, NB // 2:])
                    xta = lambda a, xtc=xtc: xtc[:, a]
                    csc = cs_pool.tile([P, 2, 2, TCH], bf, tag="cs")
                    nc.gpsimd.dma_start(csc[:], cs5[:, tcix])
                    cosc, sinc = csc[:, 0], csc[:, 1]
                if tcix == 3:
                    # wo behind the chunk-3 x stream on the sync queue:
                    # arrives long before the first output projection.
                    nc.sync.dma_start(wo_sb[:], wo[:])

                qraw = raw_pool.tile([P, 2, TCH], bf, tag="qraw")
                kraw = raw_pool.tile([P, 2, TCH], bf, tag="kraw")
                if tcix == 0:
                    # group-interleaved order: each (wq_g, xt_g, wkv_g)
                    # DMA group unlocks its matmuls immediately, so PE
                    # starts after ~1 group of traffic instead of 3 MB.
                    qk_ps = [psum.tile([P, TCH], f32, tag="qk",
                                       name=f"c0qk{j}") for j in range(4)]
                    v_ps = [psum.tile([P, 256], f32, tag="v",
                                      name=f"c0v{tt}") for tt in range(4)]
                    for g in range(NG):
                        for j, (wt, cb) in enumerate(
                                ((wq_sb, 0), (wq_sb, 1),
                                 (wk_sb, 0), (wk_sb, 1))):
                            for al in range(GA):
                                a = g * GA + al
                                nc.tensor.matmul(
                                    qk_ps[j][:],
                                    wt[g][:, al, cb * P:cb * P + P],
                                    xta(a),
                                    start=(a == 0), stop=(a == NB - 1),
                                )
                        for tt in range(TCH // P):
                            for al in range(GA):
                                a = g * GA + al
                                nc.tensor.matmul(
                                    v_ps[tt][:],
                                    xta(a)[:, tt * P:(tt + 1) * P],
                                    wv_sb[g][:, al, :],
                                    start=(a == 0), stop=(a == NB - 1),
                                )
                    for j, (rawt, cb) in enumerate(
                            ((qraw, 0), (qraw, 1), (kraw, 0), (kraw, 1))):
                        nc.scalar.activation(rawt[:, cb, :], qk_ps[j][:],
                                             Act.Copy)
                    for tt in range(TCH // P):
                        nc.scalar.activation(v_all[:, tt * 256:(tt + 1) * 256],
                                             v_ps[tt][:], Act.Copy)
                else:
                    for (wt, rawt) in ((wq_sb, qraw), (wk_sb, kraw)):
                        for cb in range(2):
                            ps = psum.tile([P, TCH], f32, tag="qk")
                            for a in range(NB):
                                nc.tensor.matmul(
                                    ps[:],
                                    wt[a // GA][:, a % GA,
                                                cb * P:cb * P + P],
                                    xta(a),
                                    start=(a == 0), stop=(a == NB - 1),
                                )
                            nc.scalar.activation(rawt[:, cb, :], ps[:],
                                                 Act.Copy)
                    # v: x-stationary, natural layout
                    for tt in range(TCH // P):
                        ps = psum.tile([P, 256], f32, tag="v")
                        for a in range(NB):
                            nc.tensor.matmul(
                                ps[:],
                                xta(a)[:, tt * P:(tt + 1) * P],
                                wv_sb[a // GA][:, a % GA, :],
                                start=(a == 0), stop=(a == NB - 1),
                            )
                        cidx = (tcix * (TCH // P) + tt) * 256
                        nc.scalar.activation(v_all[:, cidx:cidx + 256], ps[:],
                                             Act.Copy)

                # RoPE: rope_b = raw_b*cos_b + raw_{1-b}*sinT_b (sign-folded)
                for (rawt, dst) in ((qraw, q_all), (kraw, k_all)):
                    for cb in range(2):
                        tm = tmp_pool.tile([P, TCH], bf, tag="ropetmp")
                        nc.vector.tensor_mul(tm[:], rawt[:, 1 - cb, :],
                                             sinc[:, cb, :])
                        tm2 = tmp_pool.tile([P, TCH], bf, tag="ropetmp2")
                        nc.vector.tensor_mul(tm2[:], rawt[:, cb, :],
                                             cosc[:, cb, :])
                        nc.vector.tensor_add(
                            dst[:, cb * T + t0:cb * T + t0 + TCH],
                            tm[:], tm2[:])

        # ------- Phase 2+3: causal attention + output projection -------
        # scoresT blocks [kj=128, q=512]; diagonal blocks shrink to their
        # unmasked column range; exp on ScalarE; per-block causal triangle
        # via a [128,128] GpSimd mask; rowsum via ones-matmul on quad
        # presums (DVE+GpSimd); PV consumes expT directly. The inner loop
        # is software-pipelined two pairs deep (rs/pv trail sc/exp by two
        # pairs) so PE never head-of-line blocks on ScalarE's exp. The
        # output projection for a (b, qj) token group is interleaved one
        # group late, once its at_all slices are long since written.
        def attn_group(psum, b, cb, qj, depth=2):
            qs = cb * T + b * S + qj * QBLK
            nkb = 4 * qj + 4  # key blocks 0..nkb-1
            npair = nkb // 2
            nquads = nkb // 4
            pv_ps = psum.tile([P, QBLK], f32, tag="pv", bufs=1,
                              name=f"pv_{b}{cb}{qj}")
            rs_ps = psum.tile([P, QBLK], f32, tag="rs", bufs=1,
                              name=f"rs_{b}{cb}{qj}")
            quad_buf = []
            sum_tree = []  # binary-counter merge: list of (level, tile)
            counts = {"pv": 0, "quads": 0, "adds": 0}

            def blk_lo(i):
                # first unmasked column of key block i within this q block
                dd = i - 4 * qj
                return 128 * dd if dd > 0 else 0

            def tree_add(x, y, tag, bufs, eng):
                ai = counts["adds"]
                counts["adds"] += 1
                es = tmp_pool.tile([P, QBLK], bf, tag=tag,
                                   bufs=bufs, name=f"es_{b}{cb}{qj}_{ai}")
                eng.tensor_add(es[:], x, y)
                return es

            def consume(ii, ex):
                for h in range(2):
                    i = 2 * ii + h
                    lo = blk_lo(i)
                    # first matmul of the accumulation must cover the full
                    # bank (start zeroes it); masked ex columns are zero.
                    if counts["pv"] == 0:
                        lo = 0
                    vix = (b * 16 + i) * 256 + cb * P
                    nc.tensor.matmul(pv_ps[:, lo:], v_all[:, vix:vix + P],
                                     ex[:, h, lo:],
                                     start=(counts["pv"] == 0),
                                     stop=(counts["pv"] == nkb - 1))
                    counts["pv"] += 1
                # rowsum: GpSimd folds ALL exp blocks of the group into one
                # [128,512] partial-sum tile (binary-counter tree), so the
                # whole group costs a single ones-matmul on the PE. GpSimd
                # is otherwise near-idle in this phase; DVE carries the
                # PSUM evictions instead (GpSimd has no PSUM port).
                quad_buf.append(ex)
                if len(quad_buf) == 2:
                    e0, e1 = quad_buf
                    quad_buf.clear()
                    counts["quads"] += 1
                    # leaf adds split across GpSimd (ea, ~3x slower but
                    # otherwise idle) and DVE (eb) so they run in
                    # parallel; the quad sum and tree merges stay on DVE
                    # (short rs critical path at group end).
                    ea = tree_add(e0[:, 0, :], e0[:, 1, :], "esA", 2,
                                  nc.gpsimd)
                    eb = tree_add(e1[:, 0, :], e1[:, 1, :], "esB", 2,
                                  nc.vector)
                    # tree nodes outlive the quad that made them (they sit
                    # on the merge stack until an equal-level partner shows
                    # up), so they rotate through a deeper buffer set.
                    node = (1, tree_add(ea[:], eb[:], "esN", 8, nc.vector))
                    while sum_tree and sum_tree[-1][0] == node[0]:
                        lvl, prev = sum_tree.pop()
                        node = (lvl + 1, tree_add(prev[:], node[1][:],
                                                  "esN", 8, nc.vector))
                    sum_tree.append(node)

            def rowsum_flush():
                # drain the merge tree to one tile; single ones-matmul.
                assert not quad_buf
                node = sum_tree.pop()
                while sum_tree:
                    _, prev = sum_tree.pop()
                    node = (0, tree_add(prev[:], node[1][:], "esN", 8,
                                        nc.vector))
                nc.tensor.matmul(rs_ps[:], ones_sb[:], node[1][:],
                                 start=True, stop=True)

            pending = []

            def pair_step(ii, filler):
                sc_ps = psum.tile([P, 2, QBLK], f32, tag="sc",
                                  name=f"sc_{b}{cb}{qj}_{ii}")
                ex = exp_pool.tile([P, 2, QBLK], bf, tag="exp",
                                   name=f"ex_{b}{cb}{qj}_{ii}")
                los = []
                for h in range(2):
                    i = 2 * ii + h
                    lo = blk_lo(i)
                    los.append(lo)
                    ks = cb * T + b * S + i * P
                    nc.tensor.matmul(sc_ps[:, h, lo:], k_all[:, ks:ks + P],
                                     q_all[:, qs + lo:qs + QBLK],
                                     start=True, stop=True)
                # Single exp per pair, even on diagonal pairs: the h=1
                # columns [lo0:lo1) hold stale psum garbage whose exp may
                # be inf, but the memsets AFTER the activation (WAW on the
                # overlap) zero every masked column before any reader.
                if los[0] == 0 and los[1] == 0:
                    nc.scalar.activation(ex[:], sc_ps[:], Act.Exp,
                                         scale=SCALE)
                else:
                    nc.scalar.activation(ex[:, :, los[0]:],
                                         sc_ps[:, :, los[0]:],
                                         Act.Exp, scale=SCALE)
                    if los[0] > 0:
                        nc.gpsimd.memset(ex[:, 0, 0:los[0]], 0.0)
                    nc.gpsimd.memset(ex[:, 1, 0:los[1]], 0.0)
                # per-block causal triangle on the diagonal 128 columns
                for h in range(2):
                    i = 2 * ii + h
                    if i >= 4 * qj:
                        lo = los[h]
                        nc.vector.tensor_mul(ex[:, h, lo:lo + P],
                                             ex[:, h, lo:lo + P], tri_sb[:])
                # independent PE work lands here, between the exp issue and
                # the rs/pv matmuls two pairs back (PE executes in order)
                filler()
                pending.append((ii, ex))
                if len(pending) == depth + 1:
                    consume(*pending.pop(0))

            def finish(filler):
                # recip + at-mul issue before the filler's DVE evictions so
                # they sit at the head of the DVE queue: the next group's
                # first pv matmul reuses this pv bank and waits on at-mul.
                while pending:
                    consume(*pending.pop(0))
                rowsum_flush()
                rec = rec_pool.tile([P, QBLK], f32, tag="rec",
                                    name=f"rec_{b}{cb}{qj}")
                nc.vector.reciprocal_approx_fast(rec[:], rs_ps[:])
                # per-token-block at-mul: the output projection's first
                # unit starts after one [128,128] sub-mul instead of
                # waiting for the full 512-column multiply.
                for tx in range(QBLK // P):
                    nc.vector.tensor_mul(
                        at_all[:, qs + tx * P:qs + (tx + 1) * P],
                        pv_ps[:, tx * P:(tx + 1) * P],
                        rec[:, tx * P:(tx + 1) * P])
                filler()

            # diagonal pairs first: their exp -> triangle-mask chain then
            # overlaps the dense pairs' matmuls instead of the group tail.
            steps = [(lambda f, ii=ii: pair_step(ii, f))
                     for ii in range(npair - 1, -1, -1)]
            return steps, finish

        def out_units(psum, b, qj, last=False):
            # output projection for the 4 token chunks of (b, qj), split
            # into per-(token, dcol) units so they can fill PE bubbles
            # inside the next attention group's exp-chain.
            units = []

            def unit(tx, dc, orow_box):
                tt = (b * S + qj * QBLK) // P + tx
                if dc == 0:
                    orow_box.append(orow_pool.tile([P, D], bf, tag="orow",
                                                   name=f"orow_{tt}"))
                orow = orow_box[0]
                ps = psum.tile([P, 512], f32, tag="out",
                               name=f"out_{tt}_{dc}")
                for cb in range(2):
                    nc.tensor.matmul(
                        ps[:],
                        at_all[:, cb * T + tt * P:cb * T + (tt + 1) * P],
                        wo_sb[:, cb * D + dc * 512:cb * D + (dc + 1) * 512],
                        start=(cb == 0), stop=(cb == 1),
                    )
                # evictions alternate ACT/DVE; with QKV chunks 4-7
                # interleaved into the attention section, both engines
                # sit well below saturation so exp/at-mul latency stays
                # short.
                dst = orow[:, dc * 512:(dc + 1) * 512]
                if dc % 2 == 1:
                    nc.scalar.activation(dst, ps[:], Act.Copy)
                else:
                    nc.vector.tensor_copy(dst, ps[:])
                if last:
                    # final rows ship per-512-column slice, alternating
                    # over the two fast HWDGE rings (SWDGE is ~2.5x
                    # slower per byte and would set the tail), each slice
                    # as soon as its eviction lands: the post-last-matmul
                    # drain is ~1 slice per ring.
                    eng = (nc.sync, nc.scalar)[(tx + dc) % 2]
                    eng.dma_start(out[tt * P:(tt + 1) * P,
                                      dc * 512:(dc + 1) * 512], dst)
                elif dc == D // 512 - 1:
                    # steady-state output rows all on the sync ring (62
                    # GB/s demand vs ~120 capacity): keeps the gpsimd
                    # queue free for the presum-phase SWDGE traffic.
                    nc.sync.dma_start(out[tt * P:(tt + 1) * P, :], orow[:])

            for tx in range(QBLK // P):
                box = []
                for dc in range(D // 512):
                    units.append(lambda tx=tx, dc=dc, box=box: unit(tx, dc, box))
            return units

        def chunk_units(psum, tcix):
            # QKV + RoPE for one batch-1 token chunk, emitted as PE-filler
            # units inside the b=0 attention section (whose groups don't
            # depend on it). PSUM accumulators borrow the "out" tag slots.
            # Evictions on ScalarE: it only carries exp in this section.
            t0 = tcix * TCH
            xtc = xt_pool.tile([P, NB, TCH], bf, tag="xt", name=f"xt_{tcix}")
            nc.sync.dma_start(xtc[:, 0:NB // 2], xT4[:, tcix, 0:NB // 2])
            nc.scalar.dma_start(xtc[:, NB // 2:], xT4[:, tcix, NB // 2:])
            csc = cs_pool.tile([P, 2, 2, TCH], bf, tag="cs",
                               name=f"cs_{tcix}")
            nc.gpsimd.dma_start(csc[:], cs5[:, tcix])
            cosc, sinc = csc[:, 0], csc[:, 1]
            qraw = raw_pool.tile([P, 2, TCH], bf, tag="qraw",
                                 name=f"qraw_{tcix}")
            kraw = raw_pool.tile([P, 2, TCH], bf, tag="kraw",
                                 name=f"kraw_{tcix}")

            def qk_unit(wt, rawt, cb, which):
                ps = psum.tile([P, TCH], f32, tag="out",
                               name=f"c{tcix}qk_{which}{cb}")
                for a in range(NB):
                    nc.tensor.matmul(
                        ps[:], wt[a // GA][:, a % GA, cb * P:cb * P + P],
                        xtc[:, a],
                        start=(a == 0), stop=(a == NB - 1))
                nc.scalar.activation(rawt[:, cb, :], ps[:], Act.Copy)

            def v_unit(tt):
                ps = psum.tile([P, 256], f32, tag="out",
                               name=f"c{tcix}v_{tt}")
                for a in range(NB):
                    nc.tensor.matmul(
                        ps[:], xtc[:, a, tt * P:(tt + 1) * P],
                        wv_sb[a // GA][:, a % GA, :],
                        start=(a == 0), stop=(a == NB - 1))
                cidx = (tcix * (TCH // P) + tt) * 256
                nc.scalar.activation(v_all[:, cidx:cidx + 256], ps[:],
                                     Act.Copy)

            def rope_unit(rawt, dst):
                for cb in range(2):
                    tm = tmp_pool.tile([P, TCH], bf, tag="ropetmp")
                    nc.vector.tensor_mul(tm[:], rawt[:, 1 - cb, :],
                                         sinc[:, cb, :])
                    tm2 = tmp_pool.tile([P, TCH], bf, tag="ropetmp2")
                    nc.vector.tensor_mul(tm2[:], rawt[:, cb, :],
                                         cosc[:, cb, :])
                    nc.vector.tensor_add(
                        dst[:, cb * T + t0:cb * T + t0 + TCH], tm[:], tm2[:])

            return [
                lambda: qk_unit(wq_sb, qraw, 0, "q"),
                lambda: qk_unit(wq_sb, qraw, 1, "q"),
                lambda: qk_unit(wk_sb, kraw, 0, "k"),
                lambda: qk_unit(wk_sb, kraw, 1, "k"),
                lambda: v_unit(0), lambda: v_unit(1),
                lambda: v_unit(2), lambda: v_unit(3),
                lambda: rope_unit(qraw, q_all),
                lambda: rope_unit(kraw, k_all),
            ]

        with tc.tile_pool(name="psum2", bufs=2, space="PSUM") as psum:
            groups = [(b, qj) for b in range(B) for qj in range(NQ)]
            noop = lambda: None
            # each group's finish is delayed two pair-steps into the next
            # group: its pv/rs psum chain (recip, at-mul on DVE) then hides
            # behind the next group's score matmuls instead of stalling PE
            # at every group boundary (pv/rs have a single psum bank).
            fin_pend = []
            ngroups = len(groups)
            reserve = []
            pend_cu = []
            carry = []
            for gi, (b, qj) in enumerate(groups):
                emits = []
                for cb in range(2):
                    psteps, fin = attn_group(psum, b, cb, qj)
                    for si, st in enumerate(psteps):
                        emits.append(st)
                        if si == 1 and fin_pend:
                            emits.append(fin_pend.pop(0))
                    fin_pend.append(fin)
                new_units = (out_units(psum, *groups[gi - 1])
                             if gi >= 1 else [])
                if gi == ngroups - 1:
                    # hold back part of the previous group's projection as
                    # PE filler for the final fin drain below, which has
                    # no following group to hide its DVE chain behind.
                    reserve = new_units[12:]
                    new_units = new_units[:12]
                outs = carry + new_units
                if gi < 8:
                    # batch-1 QKV chunks as filler through the b=0 groups:
                    # chunk 4+gi//2's units are created at each even gi
                    # (lazy, so each DMA issues only once its xt buffer
                    # rotation slot is free) and spread over two groups.
                    if gi % 2 == 0:
                        pend_cu = chunk_units(psum, 4 + gi // 2)
                        outs = outs + pend_cu[:5]
                    else:
                        outs = outs + pend_cu[5:]
                # fillers start at position 4: after the previous group's
                # delayed finish (position 2) has written its at_all slice
                # -- except gi 0, whose fillers (chunk-4 QKV) have no
                # at_all dependency and can start immediately.
                lead = 0 if gi == 0 else 4
                k = 0
                n = len(emits)
                for ei, fn in enumerate(emits):
                    if ei < lead or not outs:
                        fn(noop)
                        continue
                    tgt = (ei - lead + 1) * len(outs) // (n - lead)

                    def filler(tgt=tgt, outs=outs):
                        nonlocal k
                        while k < tgt:
                            outs[k]()
                            k += 1
                    fn(filler)
                # flush down to the carry cap: instead of clumping the
                # leftover units (and their evictions) at the group
                # boundary -- where the burst of copies delays the next
                # group's exp/at-mul -- up to 6 spill into the next group.
                cap = 0 if gi == ngroups - 1 else 6
                while outs and k < max(0, len(outs) - cap):
                    outs[k]()
                    k += 1
                carry = outs[k:]
            # final fin drain (one fin left: group 15 cb1): reserved units
            # of group 14 keep the PE fed while its DVE finish-chain
            # (rowsum flush, recip, at-mul) runs.
            assert len(fin_pend) == 1
            fin_pend[0](lambda: [u() for u in reserve])
            for u in out_units(psum, *groups[-1], last=True):
                u()

    nc.compile()
    return nc


def _host_prep(x, cos, sin, Wq, Wk, Wv, Wo):
    """Build per-core input maps (numpy, bf16 on-device dtypes)."""
    def pblock(arr, nblk):
        # [nblk*128, F] -> [128, nblk*F] with col = a*F + f
        nb, f = nblk, arr.shape[1]
        return np.ascontiguousarray(
            arr.reshape(nb, P, f).transpose(1, 0, 2).reshape(P, nb * f))

    x2 = np.asarray(x, np.float32).reshape(T, D)
    # chunk-major x: col = ((c*NG + g)*GA + al)*TCH + tl so each chunk's
    # per-partition row is one contiguous 16 KB run in DRAM.
    xcore = np.ascontiguousarray(x2.T)                  # [D, T]
    xr = xcore.reshape(NG, GA, P, NCH, TCH)
    xT_r = np.ascontiguousarray(
        xr.transpose(2, 3, 0, 1, 4).reshape(P, NCH * NB * TCH)).astype(BF16)

    cosn = np.asarray(cos, np.float32)
    sinn = np.asarray(sin, np.float32)
    Wqn = np.asarray(Wq, np.float32)
    Wkn = np.asarray(Wk, np.float32)
    Wvn = np.asarray(Wv, np.float32)
    Won = np.asarray(Wo, np.float32)

    # intra-block causal triangle: tri[kj, qq] = (qq >= kj)
    kj = np.arange(P)[:, None]
    qq = np.arange(P)[None, :]
    tri = (qq >= kj).astype(np.float32)

    common = {
        "xT": xT_r,
        "tri": tri.astype(BF16),
        "ones": np.ones((P, P), BF16),
    }

    in_maps = []
    for m in range(N_CORES):
        cols = np.r_[128 * m:128 * m + 128, 1024 + 128 * m:1024 + 128 * m + 128]
        wq_s = pblock(Wqn[:, cols], NB).astype(BF16)
        wk_s = pblock(Wkn[:, cols], NB).astype(BF16)
        wv_s = pblock(Wvn[:, cols], NB).astype(BF16)
        wo_s = pblock(Won[cols, :], 2).astype(BF16)
        # merged k/v weights: col = ((g*2 + w)*GA + al)*256 + c
        wkv_s = np.ascontiguousarray(
            np.stack([wk_s.reshape(P, NG, GA * 256),
                      wv_s.reshape(P, NG, GA * 256)], axis=2)
            .reshape(P, NG * 2 * GA * 256))

        ct = np.tile(cosn[:, cols].T, (1, B))          # [256, 4096]
        st = np.tile(sinn[:, cols].T, (1, B)).copy()
        st[:128] *= -1.0                               # sign-fold block0
        # merged chunk-major cos/sin: [p, c, w(cos/sin), k(cb), tl]
        c5 = ct.reshape(2, P, NCH, TCH).transpose(1, 2, 0, 3)
        s5 = st.reshape(2, P, NCH, TCH).transpose(1, 2, 0, 3)
        cs_s = np.ascontiguousarray(
            np.stack([c5, s5], axis=2).reshape(P, NCH * 4 * TCH)).astype(BF16)

        in_maps.append(dict(common, wq=wq_s, wkv=wkv_s.astype(BF16),
                            wo=wo_s, cs=cs_s))
    return in_maps


def _numpy_fallback(x, cos, sin, Wq, bq, Wk, bk, Wv, bv, Wo, bo):
    """Exact fp32 reference path (only used when bq/bk are nonzero,
    which the spec's zero-filled biases never trigger)."""
    b, s, d = x.shape
    x2 = np.asarray(x, np.float32)
    q = x2 @ Wq + bq
    k = x2 @ Wk + bk
    v = x2 @ Wv + bv

    def rope(t):
        neg = np.concatenate([-t[..., d // 2:], t[..., :d // 2]], axis=-1)
        return t * cos[:s] + neg * sin[:s]

    q = rope(q).reshape(b, s, H, HD)
    k = rope(k).reshape(b, s, H, HD)
    v = v.reshape(b, s, H, HD)
    sc = np.einsum('bqhd,bkhd->bhqk', q, k) / np.sqrt(HD)
    mask = np.tril(np.ones((s, s), bool))
    sc = np.where(mask, sc, -np.inf)
    sc -= sc.max(-1, keepdims=True)
    p = np.exp(sc)
    p /= p.sum(-1, keepdims=True)
    at = np.einsum('bhqk,bkhd->bqhd', p, v).reshape(b, s, d)
    return at @ Wo + bo


def kernel(x, cos, sin, Wq, bq, Wk, bk, Wv, bv, Wo, bo):
    global LAST_RESULTS
    from concourse.bass_utils import run_bass_kernel_spmd

    if np.any(np.asarray(bq)) or np.any(np.asarray(bk)):
        return _numpy_fallback(x, cos, sin,
                               np.asarray(Wq, np.float32), np.asarray(bq, np.float32),
                               np.asarray(Wk, np.float32), np.asarray(bk, np.float32),
                               np.asarray(Wv, np.float32), np.asarray(bv, np.float32),
                               np.asarray(Wo, np.float32), np.asarray(bo, np.float32))

    if "nc" not in _CACHE:
        _CACHE["nc"] = _build_program()
    nc = _CACHE["nc"]

    in_maps = _host_prep(x, cos, sin, Wq, Wk, Wv, Wo)
    res = run_bass_kernel_spmd(nc, in_maps, core_ids=list(range(N_CORES)))
    LAST_RESULTS = res

    acc = np.zeros((T, D), np.float32)
    for r in res.results:
        acc += r["out"].astype(np.float32)
    # v-bias and output bias: attn rows sum to 1, so bv contributes bv @ Wo.
    acc += (np.asarray(bv, np.float32) @ np.asarray(Wo, np.float32)
            + np.asarray(bo, np.float32))[None, :]
    return acc.reshape(B, S, D)



# revision 40
# speedup vs baseline: 1.1804x; 1.1804x over previous
"""Trainium2 Bass kernel for nn_Attention_14877766713476.

Causal multi-head attention with full-hidden RoPE:
  q,k,v = x@W{q,k,v} + b;  q,k = rope(q,k);  heads=16, hd=128;
  causal softmax attention;  out = attn@Wo + bo.

Sharding: tensor-parallel over heads across 8 cores. RoPE pairs hidden
column c with c +/- 1024, i.e. head h with head h+8 -- so core m owns
heads {m, m+8} and RoPE stays core-local. Each core computes its two
heads end-to-end and a partial output projection (rows of Wo); the host
sums the 8 partials.

All matmuls in bf16 with fp32 PSUM accumulation. Host pre-transposes
x -> xT (contraction dim on partitions) and pre-slices/casts weights,
so the device does zero transposes.

v3 changes over v2 (trace-driven):
  - prologue fans the first-need tiles (wq g0, xt c0 g0 halves, wkv g0)
    across all four DMA rings; first matmul ~11 us instead of ~20 us.
  - steady-state x chunks split sync+scalar, cos/sin on vector: no
    per-chunk arrival jitter.
  - attention phase: ScalarE runs ONLY exp (evictions 1-in-4 ACT /
    rest DVE, chunk-7 evictions DVE, presum tree on GpSimd), so exp
    never queues behind a copy and the trailing pv matmuls don't stall.
  - diagonal pairs use a single merged exp (stale-psum columns zeroed
    by the memsets after it); rowsum is one ones-matmul per group fed
    by a GpSimd binary-counter presum tree.
  - final fin drain keeps reserved group-14 projection units as PE
    filler; last group's output DMA ships per-512-col slice round-robin
    over all four rings.

Layouts (per core, host-prepared, all bf16 unless noted):
  xT    [128, 16*4096]  col = a*4096 + t   (d = a*128 + p, t = b*2048+s)
  wq/wk/wv [128, 16*256] col = a*256 + c   (d = a*128 + p, c in 0..255)
  wo    [128, 2*2048]   col = cb*2048 + dcol  (c = cb*128 + p)
  cosT/sinT [128, 2*4096] col = cb*4096 + t   (c = cb*128 + p; sinT block0
                           negated so rope_b = q_b*cos_b + q_{1-b}*sinT_b)
  tri   [128, 128]      tri[kj, qq] = (qq >= kj)  (intra-block causal)
  ones  [128, 128]      all ones (rowsum matmul stationary)
Output per core: out [4096, 2048] bf16 partial (this core's two heads
through Wo rows); host sums partials in fp32 and adds bv@Wo + bo.
"""

import math
from contextlib import ExitStack

import numpy as np
import ml_dtypes

N_CORES = 8
B, S, D, H = 2, 2048, 2048, 16
HD = D // H          # 128
T = B * S            # 4096
P = 128
NB = D // P          # 16 contraction blocks
NG = 4               # a-groups (DMA split granularity)
GA = NB // NG        # 4 a-blocks per group
TCH = 512            # token chunk (QKV phase free dim)
NCH = T // TCH       # 8
QBLK = 512           # query block (attention phase free dim)
NQ = S // QBLK       # 4 query blocks per (batch, head)
SCALE = 1.0 / math.sqrt(HD)

BF16 = ml_dtypes.bfloat16

_CACHE = {}
LAST_RESULTS = None


def _build_program():
    import concourse.tile as tile
    from concourse import bacc, mybir

    bf = mybir.dt.bfloat16
    f32 = mybir.dt.float32
    Act = mybir.ActivationFunctionType

    nc = bacc.Bacc("TRN2", target_bir_lowering=False, debug=False,
                   num_devices=N_CORES)

    # DRAM layouts are chunk-major so every DMA descriptor reads fully
    # contiguous per-partition rows (16 KB for x, 4 KB for cos/sin) --
    # fragmented rows cost one DMA packet per 1 KB segment and cap the
    # aggregate stream far below HBM bandwidth.
    xT = nc.dram_tensor("xT", [P, NCH * NB * TCH], bf,
                        kind="ExternalInput").ap()
    wq = nc.dram_tensor("wq", [P, NB * 256], bf, kind="ExternalInput").ap()
    wkv = nc.dram_tensor("wkv", [P, NG * 2 * GA * 256], bf,
                         kind="ExternalInput").ap()
    wo = nc.dram_tensor("wo", [P, 2 * D], bf, kind="ExternalInput").ap()
    cs = nc.dram_tensor("cs", [P, NCH * 4 * TCH], bf,
                        kind="ExternalInput").ap()
    tri = nc.dram_tensor("tri", [P, P], bf, kind="ExternalInput").ap()
    ones = nc.dram_tensor("ones", [P, P], bf, kind="ExternalInput").ap()
    out = nc.dram_tensor("out", [T, D], bf, kind="ExternalOutput").ap()

    xT5 = xT.rearrange("p (c g a t) -> p c g a t", c=NCH, g=NG, a=GA)
    xT4 = xT.rearrange("p (c b t) -> p c b t", c=NCH, b=NB)
    wq4 = wq.rearrange("p (g a c) -> p g a c", g=NG, a=GA)
    wkv5 = wkv.rearrange("p (g w a c) -> p g w a c", g=NG, w=2, a=GA)
    cs5 = cs.rearrange("p (c w k t) -> p c w k t", c=NCH, w=2, k=2)

    with tile.TileContext(nc) as tc, ExitStack() as ctx:
        const = ctx.enter_context(tc.tile_pool(name="const", bufs=1))
        persist = ctx.enter_context(tc.tile_pool(name="persist", bufs=1))
        xt_pool = ctx.enter_context(tc.tile_pool(name="xt", bufs=2))
        cs_pool = ctx.enter_context(tc.tile_pool(name="cs", bufs=2))
        raw_pool = ctx.enter_context(tc.tile_pool(name="raw", bufs=2))
        tmp_pool = ctx.enter_context(tc.tile_pool(name="tmp", bufs=4))
        exp_pool = ctx.enter_context(tc.tile_pool(name="exp", bufs=6))
        rec_pool = ctx.enter_context(tc.tile_pool(name="rec", bufs=2))
        orow_pool = ctx.enter_context(tc.tile_pool(name="orow", bufs=3))

        # Weight/x prologue: the first q accumulation chain needs only
        # wq g0 + xt c0 g0, so those split across four rings (sync,
        # scalar, vector, gpsimd) and everything else follows in
        # need-order. With ~120 GB/s per ring, 4-way fan-out gets the
        # first matmul issued ~2 us after the rings go live instead of
        # serializing 768 KB behind one queue.
        wq_sb = [const.tile([P, GA, 256], bf, tag=f"wq{g}", name=f"wq_sb{g}")
                 for g in range(NG)]
        wkv_sb = [const.tile([P, 2, GA, 256], bf, tag=f"wkv{g}",
                             name=f"wkv_sb{g}") for g in range(NG)]
        wk_sb = [wkv_sb[g][:, 0] for g in range(NG)]
        wv_sb = [wkv_sb[g][:, 1] for g in range(NG)]
        # the very first accumulation step needs only wq g0's a0 column
        # block and xt c0 a0-a1, so those lead their queues; the sync
        # queue goes live ~1.5 us before scalar, gpsimd last.
        xt_c0 = xt_pool.tile([P, NB, TCH], bf, tag="xt", name="xt_0")
        nc.sync.dma_start(wq_sb[0][:, 0:1], wq4[:, 0, 0:1])
        nc.scalar.dma_start(wq_sb[0][:, 1:GA], wq4[:, 0, 1:GA])
        nc.sync.dma_start(xt_c0[:, 0:2, :], xT5[:, 0, 0, 0:2])
        nc.gpsimd.dma_start(wkv_sb[0][:], wkv5[:, 0])
        nc.scalar.dma_start(xt_c0[:, 2:GA, :], xT5[:, 0, 0, 2:GA])
        nc.sync.dma_start(xt_c0[:, GA:2 * GA, :], xT5[:, 0, 1])
        nc.scalar.dma_start(wq_sb[1][:], wq4[:, 1])
        nc.gpsimd.dma_start(wkv_sb[1][:], wkv5[:, 1])
        nc.scalar.dma_start(xt_c0[:, 2 * GA:3 * GA, :], xT5[:, 0, 2])
        nc.sync.dma_start(wq_sb[2][:], wq4[:, 2])
        nc.gpsimd.dma_start(wkv_sb[2][:], wkv5[:, 2])
        nc.sync.dma_start(xt_c0[:, 3 * GA:4 * GA, :], xT5[:, 0, 3])
        nc.scalar.dma_start(wq_sb[3][:], wq4[:, 3])
        nc.gpsimd.dma_start(wkv_sb[3][:], wkv5[:, 3])

        cs0 = cs_pool.tile([P, 2, 2, TCH], bf, tag="cs", name="cs_0")
        nc.scalar.dma_start(cs0[:], cs5[:, 0])

        wo_sb = const.tile([P, 2 * D], bf, tag="wo")
        tri_sb = const.tile([P, P], bf, tag="tri")
        ones_sb = const.tile([P, P], bf, tag="ones")
        nc.gpsimd.dma_start(tri_sb[:], tri[:])
        nc.gpsimd.dma_start(ones_sb[:], ones[:])

        # persistent activations
        q_all = persist.tile([P, 2 * T], bf, tag="q_all")      # roped qT
        k_all = persist.tile([P, 2 * T], bf, tag="k_all")      # roped kT
        v_all = persist.tile([P, 32 * 256], bf, tag="v_all")   # v natural
        at_all = persist.tile([P, 2 * T], bf, tag="at_all")    # attnT

        # ---------------- Phase 1: QKV projections + RoPE ----------------
        # Only chunks 0-3 (batch 0) run as a dedicated phase; chunks 4-7
        # are emitted as PE-filler units inside the b=0 attention groups
        # below. That stretches the attention phase's ScalarE/DVE work
        # (exp, evictions, presums) over a ~220 us window instead of
        # ~140 us, so neither engine saturates and the PE never waits.
        with tc.tile_pool(name="psum1", bufs=4, space="PSUM") as psum:
            for tcix in range(4):
                t0 = tcix * TCH
                if tcix == 0:
                    xta = lambda a: xt_c0[:, a]
                    cosc, sinc = cs0[:, 0], cs0[:, 1]
                else:
                    # steady-state chunk stream split over both HWDGE
                    # rings (sync + scalar) with cos/sin on the gpsimd
                    # SWDGE ring: each ring carries ~half the 122 GB/s
                    # demand, so per-chunk arrival jitter never stalls
                    # the PE.
                    xtc = xt_pool.tile([P, NB, TCH], bf, tag="xt",
                                       name=f"xt_{tcix}")
                    nc.sync.dma_start(xtc[:, 0:NB // 2], xT4[:, tcix, 0:NB // 2])
                    nc.scalar.dma_start(xtc[:, NB // 2:], xT4[:, tcix, NB // 2:])
                    xta = lambda a, xtc=xtc: xtc[:, a]
                    csc = cs_pool.tile([P, 2, 2, TCH], bf, tag="cs")
                    nc.gpsimd.dma_start(csc[:], cs5[:, tcix])
                    cosc, sinc = csc[:, 0], csc[:, 1]
                if tcix == 3:
                    # wo behind the chunk-3 x stream on the sync queue:
                    # arrives long before the first output projection.
                    nc.sync.dma_start(wo_sb[:], wo[:])

                qraw = raw_pool.tile([P, 2, TCH], bf, tag="qraw")
                kraw = raw_pool.tile([P, 2, TCH], bf, tag="kraw")
                if tcix == 0:
                    # group-interleaved order: each (wq_g, xt_g, wkv_g)
                    # DMA group unlocks its matmuls immediately, so PE
                    # starts after ~1 group of traffic instead of 3 MB.
                    qk_ps = [psum.tile([P, TCH], f32, tag="qk",
                                       name=f"c0qk{j}") for j in range(4)]
                    v_ps = [psum.tile([P, 256], f32, tag="v",
                                      name=f"c0v{tt}") for tt in range(4)]
                    for g in range(NG):
                        for j, (wt, cb) in enumerate(
                                ((wq_sb, 0), (wq_sb, 1),
                                 (wk_sb, 0), (wk_sb, 1))):
                            for al in range(GA):
                                a = g * GA + al
                                nc.tensor.matmul(
                                    qk_ps[j][:],
                                    wt[g][:, al, cb * P:cb * P + P],
                                    xta(a),
                                    start=(a == 0), stop=(a == NB - 1),
                                )
                        for tt in range(TCH // P):
                            for al in range(GA):
                                a = g * GA + al
                                nc.tensor.matmul(
                                    v_ps[tt][:],
                                    xta(a)[:, tt * P:(tt + 1) * P],
                                    wv_sb[g][:, al, :],
                                    start=(a == 0), stop=(a == NB - 1),
                                )
                    for j, (rawt, cb) in enumerate(
                            ((qraw, 0), (qraw, 1), (kraw, 0), (kraw, 1))):
                        nc.scalar.activation(rawt[:, cb, :], qk_ps[j][:],
                                             Act.Copy)
                    for tt in range(TCH // P):
                        nc.scalar.activation(v_all[:, tt * 256:(tt + 1) * 256],
                                             v_ps[tt][:], Act.Copy)
                else:
                    for (wt, rawt) in ((wq_sb, qraw), (wk_sb, kraw)):
                        for cb in range(2):
                            ps = psum.tile([P, TCH], f32, tag="qk")
                            for a in range(NB):
                                nc.tensor.matmul(
                                    ps[:],
                                    wt[a // GA][:, a % GA,
                                                cb * P:cb * P + P],
                                    xta(a),
                                    start=(a == 0), stop=(a == NB - 1),
                                )
                            nc.scalar.activation(rawt[:, cb, :], ps[:],
                                                 Act.Copy)
                    # v: x-stationary, natural layout
                    for tt in range(TCH // P):
                        ps = psum.tile([P, 256], f32, tag="v")
                        for a in range(NB):
                            nc.tensor.matmul(
                                ps[:],
                                xta(a)[:, tt * P:(tt + 1) * P],
                                wv_sb[a // GA][:, a % GA, :],
                                start=(a == 0), stop=(a == NB - 1),
                            )
                        cidx = (tcix * (TCH // P) + tt) * 256
                        nc.scalar.activation(v_all[:, cidx:cidx + 256], ps[:],
                                             Act.Copy)

                # RoPE: rope_b = raw_b*cos_b + raw_{1-b}*sinT_b (sign-folded)
                for (rawt, dst) in ((qraw, q_all), (kraw, k_all)):
                    for cb in range(2):
                        tm = tmp_pool.tile([P, TCH], bf, tag="ropetmp")
                        nc.vector.tensor_mul(tm[:], rawt[:, 1 - cb, :],
                                             sinc[:, cb, :])
                        tm2 = tmp_pool.tile([P, TCH], bf, tag="ropetmp2")
                        nc.vector.tensor_mul(tm2[:], rawt[:, cb, :],
                                             cosc[:, cb, :])
                        nc.vector.tensor_add(
                            dst[:, cb * T + t0:cb * T + t0 + TCH],
                            tm[:], tm2[:])

        # ------- Phase 2+3: causal attention + output projection -------
        # scoresT blocks [kj=128, q=512]; diagonal blocks shrink to their
        # unmasked column range; exp on ScalarE; per-block causal triangle
        # via a [128,128] GpSimd mask; rowsum via ones-matmul on quad
        # presums (DVE+GpSimd); PV consumes expT directly. The inner loop
        # is software-pipelined two pairs deep (rs/pv trail sc/exp by two
        # pairs) so PE never head-of-line blocks on ScalarE's exp. The
        # output projection for a (b, qj) token group is interleaved one
        # group late, once its at_all slices are long since written.
        def attn_group(psum, b, cb, qj, depth=2):
            qs = cb * T + b * S + qj * QBLK
            nkb = 4 * qj + 4  # key blocks 0..nkb-1
            npair = nkb // 2
            nquads = nkb // 4
            pv_ps = psum.tile([P, QBLK], f32, tag="pv", bufs=1,
                              name=f"pv_{b}{cb}{qj}")
            rs_ps = psum.tile([P, QBLK], f32, tag="rs", bufs=1,
                              name=f"rs_{b}{cb}{qj}")
            quad_buf = []
            sum_tree = []  # binary-counter merge: list of (level, tile)
            counts = {"pv": 0, "quads": 0, "adds": 0}

            def blk_lo(i):
                # first unmasked column of key block i within this q block
                dd = i - 4 * qj
                return 128 * dd if dd > 0 else 0

            def tree_add(x, y, tag, bufs, eng):
                ai = counts["adds"]
                counts["adds"] += 1
                es = tmp_pool.tile([P, QBLK], bf, tag=tag,
                                   bufs=bufs, name=f"es_{b}{cb}{qj}_{ai}")
                eng.tensor_add(es[:], x, y)
                return es

            def consume(ii, ex):
                for h in range(2):
                    i = 2 * ii + h
                    lo = blk_lo(i)
                    # first matmul of the accumulation must cover the full
                    # bank (start zeroes it); masked ex columns are zero.
                    if counts["pv"] == 0:
                        lo = 0
                    vix = (b * 16 + i) * 256 + cb * P
                    nc.tensor.matmul(pv_ps[:, lo:], v_all[:, vix:vix + P],
                                     ex[:, h, lo:],
                                     start=(counts["pv"] == 0),
                                     stop=(counts["pv"] == nkb - 1))
                    counts["pv"] += 1
                # rowsum: GpSimd folds ALL exp blocks of the group into one
                # [128,512] partial-sum tile (binary-counter tree), so the
                # whole group costs a single ones-matmul on the PE. GpSimd
                # is otherwise near-idle in this phase; DVE carries the
                # PSUM evictions instead (GpSimd has no PSUM port).
                quad_buf.append(ex)
                if len(quad_buf) == 2:
                    e0, e1 = quad_buf
                    quad_buf.clear()
                    counts["quads"] += 1
                    # leaf adds split across GpSimd (ea, ~3x slower but
                    # otherwise idle) and DVE (eb) so they run in
                    # parallel; the quad sum and tree merges stay on DVE
                    # (short rs critical path at group end).
                    ea = tree_add(e0[:, 0, :], e0[:, 1, :], "esA", 2,
                                  nc.gpsimd)
                    eb = tree_add(e1[:, 0, :], e1[:, 1, :], "esB", 2,
                                  nc.vector)
                    # tree nodes outlive the quad that made them (they sit
                    # on the merge stack until an equal-level partner shows
                    # up), so they rotate through a deeper buffer set.
                    node = (1, tree_add(ea[:], eb[:], "esN", 8, nc.vector))
                    while sum_tree and sum_tree[-1][0] == node[0]:
                        lvl, prev = sum_tree.pop()
                        node = (lvl + 1, tree_add(prev[:], node[1][:],
                                                  "esN", 8, nc.vector))
                    sum_tree.append(node)

            def rowsum_flush():
                # drain the merge tree to one tile; single ones-matmul.
                assert not quad_buf
                node = sum_tree.pop()
                while sum_tree:
                    _, prev = sum_tree.pop()
                    node = (0, tree_add(prev[:], node[1][:], "esN", 8,
                                        nc.vector))
                nc.tensor.matmul(rs_ps[:], ones_sb[:], node[1][:],
                                 start=True, stop=True)

            pending = []

            def pair_step(ii, filler):
                sc_ps = psum.tile([P, 2, QBLK], f32, tag="sc",
                                  name=f"sc_{b}{cb}{qj}_{ii}")
                ex = exp_pool.tile([P, 2, QBLK], bf, tag="exp",
                                   name=f"ex_{b}{cb}{qj}_{ii}")
                los = []
                for h in range(2):
                    i = 2 * ii + h
                    lo = blk_lo(i)
                    los.append(lo)
                    ks = cb * T + b * S + i * P
                    nc.tensor.matmul(sc_ps[:, h, lo:], k_all[:, ks:ks + P],
                                     q_all[:, qs + lo:qs + QBLK],
                                     start=True, stop=True)
                # Single exp per pair, even on diagonal pairs: the h=1
                # columns [lo0:lo1) hold stale psum garbage whose exp may
                # be inf, but the memsets AFTER the activation (WAW on the
                # overlap) zero every masked column before any reader.
                if los[0] == 0 and los[1] == 0:
                    nc.scalar.activation(ex[:], sc_ps[:], Act.Exp,
                                         scale=SCALE)
                else:
                    nc.scalar.activation(ex[:, :, los[0]:],
                                         sc_ps[:, :, los[0]:],
                                         Act.Exp, scale=SCALE)
                    if los[0] > 0:
                        nc.gpsimd.memset(ex[:, 0, 0:los[0]], 0.0)
                    nc.gpsimd.memset(ex[:, 1, 0:los[1]], 0.0)
                # per-block causal triangle on the diagonal 128 columns
                for h in range(2):
                    i = 2 * ii + h
                    if i >= 4 * qj:
                        lo = los[h]
                        nc.vector.tensor_mul(ex[:, h, lo:lo + P],
                                             ex[:, h, lo:lo + P], tri_sb[:])
                # independent PE work lands here, between the exp issue and
                # the rs/pv matmuls two pairs back (PE executes in order)
                filler()
                pending.append((ii, ex))
                if len(pending) == depth + 1:
                    consume(*pending.pop(0))

            def finish(filler):
                # recip + at-mul issue before the filler's DVE evictions so
                # they sit at the head of the DVE queue: the next group's
                # first pv matmul reuses this pv bank and waits on at-mul.
                while pending:
                    consume(*pending.pop(0))
                rowsum_flush()
                rec = rec_pool.tile([P, QBLK], f32, tag="rec",
                                    name=f"rec_{b}{cb}{qj}")
                nc.vector.reciprocal_approx_fast(rec[:], rs_ps[:])
                # per-token-block at-mul: the output projection's first
                # unit starts after one [128,128] sub-mul instead of
                # waiting for the full 512-column multiply.
                for tx in range(QBLK // P):
                    nc.vector.tensor_mul(
                        at_all[:, qs + tx * P:qs + (tx + 1) * P],
                        pv_ps[:, tx * P:(tx + 1) * P],
                        rec[:, tx * P:(tx + 1) * P])
                filler()

            # diagonal pairs first: their exp -> triangle-mask chain then
            # overlaps the dense pairs' matmuls instead of the group tail.
            steps = [(lambda f, ii=ii: pair_step(ii, f))
                     for ii in range(npair - 1, -1, -1)]
            return steps, finish

        def out_units(psum, b, qj, last=False):
            # output projection for the 4 token chunks of (b, qj), split
            # into per-(token, dcol) units so they can fill PE bubbles
            # inside the next attention group's exp-chain.
            units = []

            def unit(tx, dc, orow_box):
                tt = (b * S + qj * QBLK) // P + tx
                if dc == 0:
                    orow_box.append(orow_pool.tile([P, D], bf, tag="orow",
                                                   name=f"orow_{tt}"))
                orow = orow_box[0]
                ps = psum.tile([P, 512], f32, tag="out",
                               name=f"out_{tt}_{dc}")
                for cb in range(2):
                    nc.tensor.matmul(
                        ps[:],
                        at_all[:, cb * T + tt * P:cb * T + (tt + 1) * P],
                        wo_sb[:, cb * D + dc * 512:cb * D + (dc + 1) * 512],
                        start=(cb == 0), stop=(cb == 1),
                    )
                # evictions alternate ACT/DVE; with QKV chunks 4-7
                # interleaved into the attention section, both engines
                # sit well below saturation so exp/at-mul latency stays
                # short.
                dst = orow[:, dc * 512:(dc + 1) * 512]
                if dc % 2 == 1:
                    nc.scalar.activation(dst, ps[:], Act.Copy)
                else:
                    nc.vector.tensor_copy(dst, ps[:])
                if last:
                    # final rows ship per-512-column slice, alternating
                    # over the two fast HWDGE rings (SWDGE is ~2.5x
                    # slower per byte and would set the tail), each slice
                    # as soon as its eviction lands: the post-last-matmul
                    # drain is ~1 slice per ring.
                    eng = (nc.sync, nc.scalar)[(tx + dc) % 2]
                    eng.dma_start(out[tt * P:(tt + 1) * P,
                                      dc * 512:(dc + 1) * 512], dst)
                elif dc == D // 512 - 1:
                    # steady-state output rows rotate over three rings:
                    # the row stream peaks near 150 GB/s, more than any
                    # single queue sustains (HWDGE ~120, SWDGE ~60), and
                    # a backed-up ring turns into orow WAR stalls on the
                    # PE. Scalar carries only 1 in 4 (its trigger costs
                    # ~0.7 us of ScalarE queue time).
                    eng = (nc.sync, nc.scalar, nc.sync, nc.gpsimd)[tx % 4]
                    eng.dma_start(out[tt * P:(tt + 1) * P, :], orow[:])

            for tx in range(QBLK // P):
                box = []
                for dc in range(D // 512):
                    units.append(lambda tx=tx, dc=dc, box=box: unit(tx, dc, box))
            return units

        def chunk_units(psum, tcix):
            # QKV + RoPE for one batch-1 token chunk, emitted as PE-filler
            # units inside the b=0 attention section (whose groups don't
            # depend on it). PSUM accumulators borrow the "out" tag slots.
            # Evictions on ScalarE: it only carries exp in this section.
            t0 = tcix * TCH
            xtc = xt_pool.tile([P, NB, TCH], bf, tag="xt", name=f"xt_{tcix}")
            nc.sync.dma_start(xtc[:, 0:NB // 2], xT4[:, tcix, 0:NB // 2])
            nc.scalar.dma_start(xtc[:, NB // 2:], xT4[:, tcix, NB // 2:])
            csc = cs_pool.tile([P, 2, 2, TCH], bf, tag="cs",
                               name=f"cs_{tcix}")
            nc.gpsimd.dma_start(csc[:], cs5[:, tcix])
            cosc, sinc = csc[:, 0], csc[:, 1]
            qraw = raw_pool.tile([P, 2, TCH], bf, tag="qraw",
                                 name=f"qraw_{tcix}")
            kraw = raw_pool.tile([P, 2, TCH], bf, tag="kraw",
                                 name=f"kraw_{tcix}")

            def qk_unit(wt, rawt, cb, which):
                ps = psum.tile([P, TCH], f32, tag="out",
                               name=f"c{tcix}qk_{which}{cb}")
                for a in range(NB):
                    nc.tensor.matmul(
                        ps[:], wt[a // GA][:, a % GA, cb * P:cb * P + P],
                        xtc[:, a],
                        start=(a == 0), stop=(a == NB - 1))
                nc.scalar.activation(rawt[:, cb, :], ps[:], Act.Copy)

            def v_unit(tt):
                ps = psum.tile([P, 256], f32, tag="out",
                               name=f"c{tcix}v_{tt}")
                for a in range(NB):
                    nc.tensor.matmul(
                        ps[:], xtc[:, a, tt * P:(tt + 1) * P],
                        wv_sb[a // GA][:, a % GA, :],
                        start=(a == 0), stop=(a == NB - 1))
                cidx = (tcix * (TCH // P) + tt) * 256
                nc.scalar.activation(v_all[:, cidx:cidx + 256], ps[:],
                                     Act.Copy)

            def rope_unit(rawt, dst):
                for cb in range(2):
                    tm = tmp_pool.tile([P, TCH], bf, tag="ropetmp")
                    nc.vector.tensor_mul(tm[:], rawt[:, 1 - cb, :],
                                         sinc[:, cb, :])
                    tm2 = tmp_pool.tile([P, TCH], bf, tag="ropetmp2")
                    nc.vector.tensor_mul(tm2[:], rawt[:, cb, :],
                                         cosc[:, cb, :])
                    nc.vector.tensor_add(
                        dst[:, cb * T + t0:cb * T + t0 + TCH], tm[:], tm2[:])

            return [
                lambda: qk_unit(wq_sb, qraw, 0, "q"),
                lambda: qk_unit(wq_sb, qraw, 1, "q"),
                lambda: qk_unit(wk_sb, kraw, 0, "k"),
                lambda: qk_unit(wk_sb, kraw, 1, "k"),
                lambda: v_unit(0), lambda: v_unit(1),
                lambda: v_unit(2), lambda: v_unit(3),
                lambda: rope_unit(qraw, q_all),
                lambda: rope_unit(kraw, k_all),
            ]

        with tc.tile_pool(name="psum2", bufs=2, space="PSUM") as psum:
            # b0/b1 groups interleave (b1 qj needs only chunk 4+qj, which
            # the chunk-unit schedule guarantees) so exp/eviction/presum
            # load spreads uniformly over the whole section instead of
            # piling up in a dense all-b1 tail.
            groups = [(0, 0), (0, 1), (1, 0), (0, 2),
                      (1, 1), (0, 3), (1, 2), (1, 3)]
            # chunk c is created at slot CU_CREATE[c-4] (xt buffer
            # rotation: c+2's slot is after chunk c's units all ran) and
            # CU_TAKE[slot] units run per slot.
            CU_CREATE = {0: 4, 1: 5, 2: 6, 4: 7}
            CU_TAKE = [7, 7, 6, 7, 6, 7]
            noop = lambda: None
            # each group's finish is delayed two pair-steps into the next
            # group: its pv/rs psum chain (recip, at-mul on DVE) then hides
            # behind the next group's score matmuls instead of stalling PE
            # at every group boundary (pv/rs have a single psum bank).
            fin_pend = []
            ngroups = len(groups)
            reserve = []
            pend_cu = []
            carry = []
            for gi, (b, qj) in enumerate(groups):
                emits = []
                for cb in range(2):
                    psteps, fin = attn_group(psum, b, cb, qj)
                    for si, st in enumerate(psteps):
                        emits.append(st)
                        if si == 1 and fin_pend:
                            emits.append(fin_pend.pop(0))
                    fin_pend.append(fin)
                new_units = (out_units(psum, *groups[gi - 1])
                             if gi >= 1 else [])
                if gi == ngroups - 1:
                    # hold back part of the previous group's projection as
                    # PE filler for the final fin drain below, which has
                    # no following group to hide its DVE chain behind.
                    reserve = new_units[12:]
                    new_units = new_units[:12]
                # chunk units go FIRST and are never carried across slots:
                # a deferred chunk unit behind a group that needs its
                # k_all/v_all would deadlock the in-order PE queue.
                outs = carry + new_units
                if gi in CU_CREATE:
                    pend_cu.extend(chunk_units(psum, CU_CREATE[gi]))
                if gi < len(CU_TAKE):
                    outs = pend_cu[:CU_TAKE[gi]] + outs
                    del pend_cu[:CU_TAKE[gi]]
                # fillers start at position 4: after the previous group's
                # delayed finish (position 2) has written its at_all slice
                # -- except gi 0, whose fillers (chunk-4 QKV) have no
                # at_all dependency and can start immediately.
                lead = 0 if gi == 0 else 4
                k = 0
                n = len(emits)
                for ei, fn in enumerate(emits):
                    if ei < lead or not outs:
                        fn(noop)
                        continue
                    tgt = (ei - lead + 1) * len(outs) // (n - lead)

                    def filler(tgt=tgt, outs=outs):
                        nonlocal k
                        while k < tgt:
                            outs[k]()
                            k += 1
                    fn(filler)
                # flush down to the carry cap: instead of clumping the
                # leftover units (and their evictions) at the group
                # boundary -- where the burst of copies delays the next
                # group's exp/at-mul -- up to 6 projection units spill
                # into the next slot (never at gi 0, where outs holds
                # only chunk units, nor at the last slot).
                cap = 6 if 0 < gi < ngroups - 1 else 0
                while outs and k < max(0, len(outs) - cap):
                    outs[k]()
                    k += 1
                carry = outs[k:]
            # final fin drain (one fin left: group 15 cb1): reserved units
            # of group 14 keep the PE fed while its DVE finish-chain
            # (rowsum flush, recip, at-mul) runs.
            assert len(fin_pend) == 1
            fin_pend[0](lambda: [u() for u in reserve])
            for u in out_units(psum, *groups[-1], last=True):
                u()

    nc.compile()
    return nc


def _host_prep(x, cos, sin, Wq, Wk, Wv, Wo):
    """Build per-core input maps (numpy, bf16 on-device dtypes)."""
    def pblock(arr, nblk):
        # [nblk*128, F] -> [128, nblk*F] with col = a*F + f
        nb, f = nblk, arr.shape[1]
        return np.ascontiguousarray(
            arr.reshape(nb, P, f).transpose(1, 0, 2).reshape(P, nb * f))

    x2 = np.asarray(x, np.float32).reshape(T, D)
    # chunk-major x: col = ((c*NG + g)*GA + al)*TCH + tl so each chunk's
    # per-partition row is one contiguous 16 KB run in DRAM.
    xcore = np.ascontiguousarray(x2.T)                  # [D, T]
    xr = xcore.reshape(NG, GA, P, NCH, TCH)
    xT_r = np.ascontiguousarray(
        xr.transpose(2, 3, 0, 1, 4).reshape(P, NCH * NB * TCH)).astype(BF16)

    cosn = np.asarray(cos, np.float32)
    sinn = np.asarray(sin, np.float32)
    Wqn = np.asarray(Wq, np.float32)
    Wkn = np.asarray(Wk, np.float32)
    Wvn = np.asarray(Wv, np.float32)
    Won = np.asarray(Wo, np.float32)

    # intra-block causal triangle: tri[kj, qq] = (qq >= kj)
    kj = np.arange(P)[:, None]
    qq = np.arange(P)[None, :]
    tri = (qq >= kj).astype(np.float32)

    common = {
        "xT": xT_r,
        "tri": tri.astype(BF16),
        "ones": np.ones((P, P), BF16),
    }

    in_maps = []
    for m in range(N_CORES):
        cols = np.r_[128 * m:128 * m + 128, 1024 + 128 * m:1024 + 128 * m + 128]
        wq_s = pblock(Wqn[:, cols], NB).astype(BF16)
        wk_s = pblock(Wkn[:, cols], NB).astype(BF16)
        wv_s = pblock(Wvn[:, cols], NB).astype(BF16)
        wo_s = pblock(Won[cols, :], 2).astype(BF16)
        # merged k/v weights: col = ((g*2 + w)*GA + al)*256 + c
        wkv_s = np.ascontiguousarray(
            np.stack([wk_s.reshape(P, NG, GA * 256),
                      wv_s.reshape(P, NG, GA * 256)], axis=2)
            .reshape(P, NG * 2 * GA * 256))

        ct = np.tile(cosn[:, cols].T, (1, B))          # [256, 4096]
        st = np.tile(sinn[:, cols].T, (1, B)).copy()
        st[:128] *= -1.0                               # sign-fold block0
        # merged chunk-major cos/sin: [p, c, w(cos/sin), k(cb), tl]
        c5 = ct.reshape(2, P, NCH, TCH).transpose(1, 2, 0, 3)
        s5 = st.reshape(2, P, NCH, TCH).transpose(1, 2, 0, 3)
        cs_s = np.ascontiguousarray(
            np.stack([c5, s5], axis=2).reshape(P, NCH * 4 * TCH)).astype(BF16)

        in_maps.append(dict(common, wq=wq_s, wkv=wkv_s.astype(BF16),
                            wo=wo_s, cs=cs_s))
    return in_maps


def _numpy_fallback(x, cos, sin, Wq, bq, Wk, bk, Wv, bv, Wo, bo):
    """Exact fp32 reference path (only used when bq/bk are nonzero,
    which the spec's zero-filled biases never trigger)."""
    b, s, d = x.shape
    x2 = np.asarray(x, np.float32)
    q = x2 @ Wq + bq
    k = x2 @ Wk + bk
    v = x2 @ Wv + bv

    def rope(t):
        neg = np.concatenate([-t[..., d // 2:], t[..., :d // 2]], axis=-1)
        return t * cos[:s] + neg * sin[:s]

    q = rope(q).reshape(b, s, H, HD)
    k = rope(k).reshape(b, s, H, HD)
    v = v.reshape(b, s, H, HD)
    sc = np.einsum('bqhd,bkhd->bhqk', q, k) / np.sqrt(HD)
    mask = np.tril(np.ones((s, s), bool))
    sc = np.where(mask, sc, -np.inf)
    sc -= sc.max(-1, keepdims=True)
    p = np.exp(sc)
    p /= p.sum(-1, keepdims=True)
    at = np.einsum('bhqk,bkhd->bqhd', p, v).reshape(b, s, d)
    return at @ Wo + bo


def kernel(x, cos, sin, Wq, bq, Wk, bk, Wv, bv, Wo, bo):
    global LAST_RESULTS
    from concourse.bass_utils import run_bass_kernel_spmd

    if np.any(np.asarray(bq)) or np.any(np.asarray(bk)):
        return _numpy_fallback(x, cos, sin,
                               np.asarray(Wq, np.float32), np.asarray(bq, np.float32),
                               np.asarray(Wk, np.float32), np.asarray(bk, np.float32),
                               np.asarray(Wv, np.float32), np.asarray(bv, np.float32),
                               np.asarray(Wo, np.float32), np.asarray(bo, np.float32))

    if "nc" not in _CACHE:
        _CACHE["nc"] = _build_program()
    nc = _CACHE["nc"]

    in_maps = _host_prep(x, cos, sin, Wq, Wk, Wv, Wo)
    res = run_bass_kernel_spmd(nc, in_maps, core_ids=list(range(N_CORES)))
    LAST_RESULTS = res

    acc = np.zeros((T, D), np.float32)
    for r in res.results:
        acc += r["out"].astype(np.float32)
    # v-bias and output bias: attn rows sum to 1, so bv contributes bv @ Wo.
    acc += (np.asarray(bv, np.float32) @ np.asarray(Wo, np.float32)
            + np.asarray(bo, np.float32))[None, :]
    return acc.reshape(B, S, D)

